# revision 2
# baseline (speedup 1.0000x reference)
"""Trainium2 Bass kernel for a 1-D correlation volume (stereo cost volume).

Problem: out[n, i, h, w] = (1/C) * sum_c x1[n,c,h,w] * x2[n,c,h,w-i],
zero where w-i < 0, for i in 0..D (D=64).
Shapes: x1, x2 = [8, 128, 128, 256] f32; out = [8, 65, 128, 256] f32.

Sharding: data-parallel over the batch dim — each of the 8 NeuronCores
processes one sample end to end (no collectives).

Per-core algorithm
------------------
The contraction over c maps onto the TensorEngine as a banded Gram
matmul: for each (h, w-tile ts) the matmul
    band[p, col] = (1/C) * sum_c x1[c, ts+p] * x2[c, (ts-64)+col]
holds every needed output as band[p, p + 64 - i].  Extracting those 65
diagonals cannot be done by any SBUF compute engine (per-partition
varying offsets), so the band is staged to a DRAM scratch where linear
addressing collapses the diagonal into a plain 3-dim strided DMA:
    addr(h, p, j) = h*192 + p*(H*192 + 1) + j       (j = 64 - i)
which reads back with h in the partition dim.  A small DVE repack then
reverses j -> i and transposes (p, j) -> (i, p) so the final store to
out[n, :, :, ts:ts+128] is a contiguous 3-dim DMA.
"""

import numpy as np

import concourse.bass as bass
import concourse.tile as tile
from concourse import bacc, mybir
from concourse.bass_utils import run_bass_kernel_spmd

# Problem constants (hardcoded per the harness contract).
B = 8          # batch == number of cores
C = 128        # channels (matmul K)
H = 128        # rows
W = 256        # cols
D = 64         # max disparity
ND = D + 1     # number of disparities (65)
T = 128        # w-tile size (matmul M)
NT = W // T    # 2 w-tiles
BANDC = T + D  # 192 band columns per tile
HB = 8         # h rows per load/staging block
PH = 64        # p-half size for the extraction stage

F32 = mybir.dt.float32


def _corr_body(tc, out_d, x1_d, x2_d):
    nc = tc.nc
    with (
        tc.tile_pool(name="io", bufs=2) as io_pool,
        tc.tile_pool(name="band", bufs=2) as band_pool,
        tc.tile_pool(name="psum", bufs=8, space="PSUM") as psum_pool,
        tc.tile_pool(name="fib", bufs=2) as fib_pool,
        tc.tile_pool(name="dram", bufs=1, space="DRAM") as dram_pool,
    ):
        # DRAM scratch, one band volume per w-tile: scr[p, h, col].
        scr = [
            dram_pool.tile([T, H, BANDC], F32, tag=f"scr{t}", name=f"scr{t}")
            for t in range(NT)
        ]

        for hb in range(0, H, HB):
            x1t = io_pool.tile([C, HB * W], F32, tag="x1t")
            nc.sync.dma_start(x1t[:], x1_d[:, hb : hb + HB, :])
            x2t = io_pool.tile([C, HB * W], F32, tag="x2t")
            nc.sync.dma_start(x2t[:], x2_d[:, hb : hb + HB, :])

            bb = [
                band_pool.tile([T, HB * BANDC], F32, tag=f"bb{t}", name=f"bb{t}")
                for t in range(NT)
            ]
            # ts=0 band columns 0:64 are w' < 0 -> zero padding.
            nc.gpsimd.memset(bb[0][:], 0.0)

            for hl in range(HB):
                base = hl * W
                # w-tile 0: band cols 64:192 <- x1[:, 0:128]^T @ x2[:, 0:128]
                pt0 = psum_pool.tile([T, T], F32, tag="pt")
                nc.tensor.matmul(
                    pt0[:],
                    x1t[:, base : base + T],
                    x2t[:, base : base + T],
                    start=True,
                    stop=True,
                )
                nc.scalar.mul(
                    bb[0][:, hl * BANDC + D : (hl + 1) * BANDC], pt0[:], 1.0 / C
                )
                # w-tile 1: band cols 0:192 <- x1[:, 128:256]^T @ x2[:, 64:256]
                pt1 = psum_pool.tile([T, BANDC], F32, tag="pt")
                nc.tensor.matmul(
                    pt1[:],
                    x1t[:, base + T : base + 2 * T],
                    x2t[:, base + T - D : base + W],
                    start=True,
                    stop=True,
                )
                nc.scalar.mul(
                    bb[1][:, hl * BANDC : (hl + 1) * BANDC], pt1[:], 1.0 / C
                )

            for t in range(NT):
                nc.sync.dma_start(scr[t][:, hb : hb + HB, :], bb[t][:])

        # Extraction: diagonal fibers out of the scratch, h in partitions.
        for t in range(NT):
            for ph in range(0, T, PH):
                ft = fib_pool.tile([H, PH, ND], F32, tag="ft")
                src = bass.AP(
                    scr[t].tensor,
                    scr[t].offset + ph * (H * BANDC + 1),
                    [[BANDC, H], [H * BANDC + 1, PH], [1, ND]],
                )
                nc.sync.dma_start(ft[:], src)

                gt = fib_pool.tile([H, ND, PH], F32, tag="gt")
                for j in range(ND):
                    nc.vector.tensor_copy(gt[:, D - j, :], ft[:, :, j])

                dst = bass.AP(
                    out_d,
                    t * T + ph,
                    [[W, H], [H * W, ND], [1, PH]],
                )
                nc.sync.dma_start(dst, gt[:])


_NC_CACHE = None


def _build_nc():
    global _NC_CACHE
    if _NC_CACHE is not None:
        return _NC_CACHE
    nc = bacc.Bacc("TRN2")
    x1_d = nc.declare_dram_parameter("x1", [C, H, W], F32, isOutput=False)
    x2_d = nc.declare_dram_parameter("x2", [C, H, W], F32, isOutput=False)
    out_d = nc.declare_dram_parameter("out", [ND, H, W], F32, isOutput=True)
    with tile.TileContext(nc) as tc:
        _corr_body(tc, out_d, x1_d, x2_d)
    nc.finalize()
    _NC_CACHE = nc
    return nc


NCORES = B


def _make_in_maps(x1, x2):
    return [
        {
            "x1": np.ascontiguousarray(x1[n], dtype=np.float32),
            "x2": np.ascontiguousarray(x2[n], dtype=np.float32),
        }
        for n in range(B)
    ]


def kernel(x1: np.ndarray, x2: np.ndarray) -> np.ndarray:
    assert x1.shape == (B, C, H, W) and x2.shape == (B, C, H, W)
    nc = _build_nc()
    res = run_bass_kernel_spmd(nc, _make_in_maps(x1, x2), list(range(B)))
    return np.stack([res.results[n]["out"] for n in range(B)], axis=0)



# revision 5
# speedup vs baseline: 1.0280x; 1.0280x over previous
"""Trainium2 Bass kernel for a 1-D correlation volume (stereo cost volume).

Problem: out[n, i, h, w] = (1/C) * sum_c x1[n,c,h,w] * x2[n,c,h,w-i],
zero where w-i < 0, for i in 0..D (D=64).
Shapes: x1, x2 = [8, 128, 128, 256] f32; out = [8, 65, 128, 256] f32.

Sharding: data-parallel over the batch dim — each of the 8 NeuronCores
processes one sample end to end (no collectives).

Per-core algorithm (fully on-chip, no DRAM scratch)
---------------------------------------------------
For each w-tile ts (two tiles of 128), the contraction over c is a banded
Gram matmul per row h:
    band[p, col] = (1/C) * sum_c x1[c, ts+p] * x2[c, (ts-64)+col]
holding out[i, h, ts+p] at col = p + 64 - i.  Extracting those diagonals
needs per-partition offsets, which no SBUF engine can do while p is the
partition dim.  The trick: transpose each band column slice [p, h] ->
[h, p] on the TensorEngine into tr[h, col, p] (h now on partitions).
In that layout the diagonal for disparity i sits at free offset
    col*128 + p = (64-i)*128 + p*129
-- a plain strided AP, identical on every partition.  A DVE copy per i
then yields out[h, i, p] ready for a contiguous store.  Inputs are cast
to bf16 on load (rel err ~3e-3 << the 2e-2 gate), so total HBM traffic
is the bare minimum: 33.5 MB in + 8.5 MB out per core.
"""

import numpy as np

import concourse.bass as bass
import concourse.tile as tile
from concourse import bacc, masks, mybir
from concourse.bass_utils import run_bass_kernel_spmd

# Problem constants (hardcoded per the harness contract).
B = 8          # batch == number of cores
C = 128        # channels (matmul K)
H = 128        # rows
W = 256        # cols
D = 64         # max disparity
ND = D + 1     # number of disparities (65)
T = 128        # w-tile size (matmul M)
NT = W // T    # 2 w-tiles
BANDC = T + D  # 192 band columns per tile
HB = 8         # h rows per input streaming block
IQ = 17        # disparities per output staging block (17+16+16+16)

F32 = mybir.dt.float32
BF16 = mybir.dt.bfloat16

NCORES = B


def _corr_body(tc, out_d, x1_d, x2_d):
    nc = tc.nc
    with (
        tc.tile_pool(name="io", bufs=2) as io_pool,
        tc.tile_pool(name="band", bufs=1) as band_pool,
        tc.tile_pool(name="tr", bufs=1) as tr_pool,
        tc.tile_pool(name="osb", bufs=2) as osb_pool,
        tc.tile_pool(name="single", bufs=1) as single_pool,
        tc.tile_pool(name="mm_psum", bufs=2, space="PSUM") as mm_psum,
        tc.tile_pool(name="tp_psum", bufs=4, space="PSUM") as tp_psum,
    ):
        ident = single_pool.tile([T, T], BF16, tag="ident", name="ident")
        masks.make_identity(nc, ident[:])

        # Band tiles, filled across the whole h loop.
        # bb[0] holds band cols 64:192 for w-tile 0 (cols 0:64 are zero),
        # bb[1] holds band cols 0:192 for w-tile 1.  Layout [p, col, h].
        bb0 = band_pool.tile([T, T, H], BF16, tag="bb0", name="bb0")
        bb1 = band_pool.tile([T, BANDC, H], BF16, tag="bb1", name="bb1")

        # Transposed band [h, col, p]; reused for w-tile 0 then 1.
        tr = tr_pool.tile([H, BANDC, T], BF16, tag="tr", name="tr")
        # w-tile 0 reads cols 0:64 as zeros (w-64+col < 0 region).
        nc.gpsimd.memset(tr[:, 0:D, :], 0.0)

        # ---- Stage 1: banded Gram matmuls, band staged as [p, col, h] ----
        for hb in range(0, H, HB):
            x1t = io_pool.tile([C, HB, W], BF16, tag="x1t", name="x1t")
            nc.gpsimd.dma_start(x1t[:], x1_d[:, hb : hb + HB, :])
            x2t = io_pool.tile([C, HB, W], BF16, tag="x2t", name="x2t")
            nc.gpsimd.dma_start(x2t[:], x2_d[:, hb : hb + HB, :])

            for hl in range(HB):
                h = hb + hl
                # w-tile 0: band cols 64:192 <- x1[:, 0:128]^T @ x2[:, 0:128]
                pt0 = mm_psum.tile([T, T], F32, tag="pt0", name="pt0")
                nc.tensor.matmul(
                    pt0[:],
                    x1t[:, hl, 0:T],
                    x2t[:, hl, 0:T],
                    start=True,
                    stop=True,
                )
                nc.scalar.mul(bb0[:, :, h], pt0[:], 1.0 / C)
                # w-tile 1: band cols 0:192 <- x1[:, 128:256]^T @ x2[:, 64:256]
                pt1 = mm_psum.tile([T, BANDC], F32, tag="pt1", name="pt1")
                nc.tensor.matmul(
                    pt1[:],
                    x1t[:, hl, T : 2 * T],
                    x2t[:, hl, T - D : W],
                    start=True,
                    stop=True,
                )
                nc.scalar.mul(bb1[:, :, h], pt1[:], 1.0 / C)

        # ---- Stage 2 per w-tile: transpose band cols, extract diagonals ----
        for t in range(NT):
            bb = bb0 if t == 0 else bb1
            col0 = D if t == 0 else 0  # first valid band col
            for col in range(col0, BANDC):
                ptr = tp_psum.tile([H, T], BF16, tag="ptr", name="ptr")
                nc.tensor.transpose(ptr[:], bb[:, col - col0, :], ident[:])
                nc.vector.tensor_copy(tr[:, col, :], ptr[:])

            for i0 in range(0, ND, IQ):
                ilen = min(IQ, ND - i0)
                osb = osb_pool.tile([H, IQ, T], F32, tag="osb", name="osb")
                for il in range(ilen):
                    i = i0 + il
                    diag = bass.AP(
                        tr.tensor,
                        tr.offset + (D - i) * T,
                        [[BANDC * T, H], [T + 1, T]],
                    )
                    nc.vector.tensor_copy(osb[:, il, :], diag)
                dst = bass.AP(
                    out_d,
                    i0 * H * W + t * T,
                    [[W, H], [H * W, ilen], [1, T]],
                )
                nc.sync.dma_start(dst, osb[:, 0:ilen, :])


_NC_CACHE = None


def _build_nc():
    global _NC_CACHE
    if _NC_CACHE is not None:
        return _NC_CACHE
    nc = bacc.Bacc("TRN2")
    x1_d = nc.declare_dram_parameter("x1", [C, H, W], F32, isOutput=False)
    x2_d = nc.declare_dram_parameter("x2", [C, H, W], F32, isOutput=False)
    out_d = nc.declare_dram_parameter("out", [ND, H, W], F32, isOutput=True)
    with tile.TileContext(nc) as tc:
        _corr_body(tc, out_d, x1_d, x2_d)
    nc.finalize()
    _NC_CACHE = nc
    return nc


def _make_in_maps(x1, x2):
    return [
        {
            "x1": np.ascontiguousarray(x1[n], dtype=np.float32),
            "x2": np.ascontiguousarray(x2[n], dtype=np.float32),
        }
        for n in range(B)
    ]


def kernel(x1: np.ndarray, x2: np.ndarray) -> np.ndarray:
    assert x1.shape == (B, C, H, W) and x2.shape == (B, C, H, W)
    nc = _build_nc()
    res = run_bass_kernel_spmd(nc, _make_in_maps(x1, x2), list(range(B)))
    return np.stack([res.results[n]["out"] for n in range(B)], axis=0)


# revision 10
# speedup vs baseline: 1.3601x; 1.3230x over previous
"""Trainium2 Bass kernel for a 1-D correlation volume (stereo cost volume).

Problem: out[n, i, h, w] = (1/C) * sum_c x1[n,c,h,w] * x2[n,c,h,w-i],
zero where w-i < 0, for i in 0..D (D=64).
Shapes: x1, x2 = [8, 128, 128, 256] f32; out = [8, 65, 128, 256] f32.

Sharding: data-parallel over the batch dim — each of the 8 NeuronCores
processes one sample end to end (no collectives).

Per-core algorithm (fully on-chip, no DRAM scratch)
---------------------------------------------------
For each w-tile ts (two tiles of 128), the contraction over c is a banded
Gram matmul per row h:
    band[p, col] = (1/C) * sum_c x1[c, ts+p] * x2[c, (ts-64)+col]
holding out[i, h, ts+p] at col = p + 64 - i.  Extracting those diagonals
needs per-partition offsets, which no SBUF engine can do while p is the
partition dim.  The trick: transpose each band column slice [p, h] ->
[h, p] on the TensorEngine into tr[h, col, p] (h now on partitions).
In that layout the diagonal for disparity i sits at free offset
    col*128 + p = (64-i)*128 + p*129
-- a plain strided AP, identical on every partition.  A DVE copy per i
then yields out[h, i, p] ready for a contiguous store.  Inputs are cast
to bf16 on load (rel err ~3e-3 << the 2e-2 gate), so total HBM traffic
is the bare minimum: 33.5 MB in + 8.5 MB out per core.
"""

import numpy as np

import concourse.bass as bass
import concourse.tile as tile
from concourse import bacc, masks, mybir
from concourse.bass_utils import run_bass_kernel_spmd

# Problem constants (hardcoded per the harness contract).
B = 8          # batch == number of cores
C = 128        # channels (matmul K)
H = 128        # rows
W = 256        # cols
D = 64         # max disparity
ND = D + 1     # number of disparities (65)
T = 128        # w-tile size (matmul M)
NT = W // T    # 2 w-tiles
BANDC = T + D  # 192 band columns per tile
HB = 8         # h rows per input streaming block
IQ = 17        # disparities per output staging block (17+16+16+16)

F32 = mybir.dt.float32
BF16 = mybir.dt.bfloat16

NCORES = B


def _corr_body(tc, out_d, x1_d, x2_d):
    nc = tc.nc
    with (
        tc.tile_pool(name="io", bufs=2) as io_pool,
        tc.tile_pool(name="band", bufs=1) as band_pool,
        tc.tile_pool(name="tr", bufs=1) as tr_pool,
        tc.tile_pool(name="osb", bufs=2) as osb_pool,
        tc.tile_pool(name="single", bufs=1) as single_pool,
        tc.tile_pool(name="mm_psum", bufs=2, space="PSUM") as mm_psum,
        tc.tile_pool(name="tp_psum", bufs=2, space="PSUM") as tp_psum,
    ):
        ident = single_pool.tile([T, T], BF16, tag="ident", name="ident")
        masks.make_identity(nc, ident[:])

        # Band tiles, filled across the whole h loop.
        # bb0 holds band cols 64:192 for w-tile 0 (cols 0:64 are zero),
        # bb1 holds band cols 0:192 for w-tile 1.  Layout [p, h, col] so the
        # ACT psum->SBUF copy writes contiguously; the transpose reads its
        # stationary operand at stride BANDC instead (LDW tolerates that).
        bb0 = band_pool.tile([T, H, T], BF16, tag="bb0", name="bb0")
        bb1 = band_pool.tile([T, H, BANDC], BF16, tag="bb1", name="bb1")

        # Transposed band [h, col, p]; reused for w-tile 0 then 1.
        tr = tr_pool.tile([H, BANDC, T], BF16, tag="tr", name="tr")
        # w-tile 0 reads cols 0:64 as zeros (w-64+col < 0 region).
        nc.gpsimd.memset(tr[:, 0:D, :], 0.0)

        # ---- Stage 1: banded Gram matmuls, band staged as [p, col, h] ----
        for hb in range(0, H, HB):
            x1t = io_pool.tile([C, HB, W], BF16, tag="x1t", name="x1t")
            nc.gpsimd.dma_start(x1t[:], x1_d[:, hb : hb + HB, :])
            x2t = io_pool.tile([C, HB, W], BF16, tag="x2t", name="x2t")
            nc.gpsimd.dma_start(x2t[:], x2_d[:, hb : hb + HB, :])

            for hl in range(HB):
                h = hb + hl
                # w-tile 0: band cols 64:192 <- x1[:, 0:128]^T @ x2[:, 0:128]
                pt0 = mm_psum.tile([T, T], F32, tag="pt0", name="pt0")
                nc.tensor.matmul(
                    pt0[:],
                    x1t[:, hl, 0:T],
                    x2t[:, hl, 0:T],
                    start=True,
                    stop=True,
                )
                nc.scalar.mul(bb0[:, h, :], pt0[:], 1.0 / C)
                # w-tile 1: band cols 0:192 <- x1[:, 128:256]^T @ x2[:, 64:256]
                pt1 = mm_psum.tile([T, BANDC], F32, tag="pt1", name="pt1")
                nc.tensor.matmul(
                    pt1[:],
                    x1t[:, hl, T : 2 * T],
                    x2t[:, hl, T - D : W],
                    start=True,
                    stop=True,
                )
                nc.scalar.mul(bb1[:, h, :], pt1[:], 1.0 / C)

        # ---- Stage 2 per w-tile: transpose band cols, extract diagonals ----
        CG = 8  # band cols per PSUM bank / per DVE drain copy
        for t in range(NT):
            bb = bb0 if t == 0 else bb1
            col0 = D if t == 0 else 0  # first valid band col
            for cg in range(col0, BANDC, CG):
                ptr = tp_psum.tile([H, CG, T], BF16, tag="ptr", name="ptr")
                for cl in range(CG):
                    col = cg + cl
                    nc.tensor.transpose(
                        ptr[:, cl, :], bb[:, :, col - col0], ident[:]
                    )
                nc.vector.tensor_copy(tr[:, cg : cg + CG, :], ptr[:])

            for i0 in range(0, ND, IQ):
                ilen = min(IQ, ND - i0)
                osb = osb_pool.tile([H, IQ, T], F32, tag="osb", name="osb")
                # Diagonal fiber block: free offset (D-i)*T + p*(T+1), the
                # i dimension walking backwards through band columns.
                diag = bass.AP(
                    tr.tensor,
                    tr.offset + (D - i0) * T,
                    [[BANDC * T, H], [-T, ilen], [T + 1, T]],
                )
                nc.vector.tensor_copy(osb[:, 0:ilen, :], diag)
                dst = bass.AP(
                    out_d,
                    i0 * H * W + t * T,
                    [[W, H], [H * W, ilen], [1, T]],
                )
                nc.sync.dma_start(dst, osb[:, 0:ilen, :])


_NC_CACHE = None


def _build_nc():
    global _NC_CACHE
    if _NC_CACHE is not None:
        return _NC_CACHE
    nc = bacc.Bacc("TRN2")
    x1_d = nc.declare_dram_parameter("x1", [C, H, W], F32, isOutput=False)
    x2_d = nc.declare_dram_parameter("x2", [C, H, W], F32, isOutput=False)
    out_d = nc.declare_dram_parameter("out", [ND, H, W], F32, isOutput=True)
    with tile.TileContext(nc) as tc:
        _corr_body(tc, out_d, x1_d, x2_d)
    nc.finalize()
    _NC_CACHE = nc
    return nc


def _make_in_maps(x1, x2):
    return [
        {
            "x1": np.ascontiguousarray(x1[n], dtype=np.float32),
            "x2": np.ascontiguousarray(x2[n], dtype=np.float32),
        }
        for n in range(B)
    ]


def kernel(x1: np.ndarray, x2: np.ndarray) -> np.ndarray:
    assert x1.shape == (B, C, H, W) and x2.shape == (B, C, H, W)
    nc = _build_nc()
    res = run_bass_kernel_spmd(nc, _make_in_maps(x1, x2), list(range(B)))
    return np.stack([res.results[n]["out"] for n in range(B)], axis=0)
